# revision 1
# baseline (speedup 1.0000x reference)
"""Trainium2 Bass kernel for 3D-PoPE multi-head self-attention.

Sharding: pure data-parallel over batch (B=8 -> 8 cores, one batch element
per core). Weights replicated. All rotary/cache gathers precomputed on host
(tiny tensors); all matmuls/softmax on device.

Per-core device computation (S=1024, D=1024, H=16, HD=64):
  - QKV projection, q/k produced feature-major [e, s], v sequence-major [s, e]
  - mu = softplus via exp then ln(x+1) (same ACT table set as attention exp)
  - q2 = [mu*cos_t; mu*sin_t]   (feature-major [128, S] per head)
  - k2 = [mu*cos_k; mu*sin_k]
  - scoresT[j, i] = k2_tile.T @ q2  (keys on partitions -> no transposes)
  - probsT = exp(scoresT / sqrt(128)) (no max subtraction: scores bounded)
  - PV with ones-augmented v: psum rows 0:64 = attnT, row 64 = softmax denom
  - normalize via reciprocal + K=1 f32r ones-matmul broadcast
  - out = attnT.T @ w_out.T  via feature-major lhsT
"""
import math

import numpy as np
import ml_dtypes

B, S, D, H = 8, 1024, 1024, 16
HD = D // H
DX = HD // 3
DY = HD // 3
DZ = HD - DX - DY
MX, MY, MZ = 32, 32, 8
BASE = 10000.0
TWO_PI = 2.0 * math.pi
BF16 = ml_dtypes.bfloat16

NT = S // 128          # 8 sequence tiles
ND = D // 128          # 8 contraction tiles
SCALE = 1.0 / math.sqrt(2.0 * HD)


def _host_prep(hidden_states, pos_xyz, w_qkv, w_out, phase_bias):
    """Host-side: transposes, dtype casts, rotary cache gather."""
    def cache(dim, maxp):
        inv = 1.0 / (BASE ** (np.arange(dim, dtype=np.float64) / dim))
        t = np.arange(maxp, dtype=np.float64)[:, None] * inv[None, :]
        return np.cos(t), np.sin(t)

    cx, sx = cache(DX, MX)
    cy, sy = cache(DY, MY)
    cz, sz = cache(DZ, MZ)
    pos = np.asarray(pos_xyz)
    px = np.clip(pos[..., 0], 0, MX - 1).astype(np.int64)
    py = np.clip(pos[..., 1], 0, MY - 1).astype(np.int64)
    pz = np.clip(pos[..., 2], 0, MZ - 1).astype(np.int64)
    cos_t = np.concatenate([cx[px], cy[py], cz[pz]], axis=-1)  # [B,S,HD] f64
    sin_t = np.concatenate([sx[px], sy[py], sz[pz]], axis=-1)
    bias = np.clip(np.asarray(phase_bias, np.float64), -TWO_PI, 0.0)
    cos_b = np.cos(bias)
    sin_b = np.sin(bias)
    cos_k = cos_t * cos_b - sin_t * sin_b
    sin_k = sin_t * cos_b + cos_t * sin_b

    def dup(x):
        # [B,S,HD] -> [B, 128, S] bf16 with rows 0:64 == rows 64:128
        xt = np.ascontiguousarray(x.transpose(0, 2, 1))  # [B, HD, S]
        return np.concatenate([xt, xt], axis=1).astype(BF16)

    hs = np.asarray(hidden_states, np.float32)
    hsT = np.ascontiguousarray(hs.transpose(0, 2, 1)).astype(BF16)  # [B, D, S]
    wqkvT = np.ascontiguousarray(np.asarray(w_qkv, np.float32).T).astype(BF16)
    woutT = np.ascontiguousarray(np.asarray(w_out, np.float32).T).astype(BF16)
    return hsT, wqkvT, woutT, dup(cos_t), dup(sin_t), dup(cos_k), dup(sin_k)


def _emit(tc, nc, t_hsT, t_wqkvT, t_woutT, t_rot, t_out,
          skip_attn=False, attn_only=False):
    import concourse.mybir as mybir

    dt = mybir.dt
    AF = mybir.ActivationFunctionType
    f32 = dt.float32
    bf = dt.bfloat16

    with (
        tc.tile_pool(name="persist", bufs=1) as pp,
        tc.tile_pool(name="work", bufs=1) as wp,
    ):
        # ---------------- load phase ----------------
        rot = pp.tile([128, 4 * S], bf, tag="rot", bufs=1)
        nc.sync.dma_start(rot[:], t_rot[:])
        r_cq = rot[:, 0:S]
        r_sq = rot[:, S:2 * S]
        r_ck = rot[:, 2 * S:3 * S]
        r_sk = rot[:, 3 * S:4 * S]

        vaug = pp.tile([128, NT * H * 65], bf, tag="vaug", bufs=1)
        va_r = vaug.rearrange("p (k h c) -> p k h c", k=NT, h=H)
        nc.vector.memset(va_r[:, :, :, 64:65], 1.0)

        ones_f = wp.tile([1, 64], f32, tag="ones_f", bufs=1)
        nc.vector.memset(ones_f[:], 1.0)
        ones_r = wp.tile([1, 64], dt.float32r, tag="ones_r", bufs=1)
        nc.vector.tensor_copy(ones_r[:], ones_f[:])

        with (
            tc.tile_pool(name="proj", bufs=1) as jp,
            tc.tile_pool(name="psum_proj", bufs=1, space="PSUM") as psp,
        ):
            hsT = jp.tile([128, ND * S], bf, tag="hsT", bufs=1)
            for i in range(ND):
                nc.sync.dma_start(
                    hsT[:, i * S:(i + 1) * S], t_hsT[i * 128:(i + 1) * 128, :])

            def load_w(col0):
                w = jp.tile([128, ND * D], bf, tag="wq", bufs=2)
                for i in range(ND):
                    nc.sync.dma_start(
                        w[:, i * D:(i + 1) * D],
                        t_wqkvT[i * 128:(i + 1) * 128, col0:col0 + D])
                return w

            # ---------------- v projection ----------------
            wv = load_w(2 * D)
            for st in range(NT if not attn_only else 0):
                ps = psp.tile([128, D], f32, tag="ps_proj", bufs=2)
                for di in range(ND):
                    lhsT = hsT[:, di * S + st * 128:di * S + (st + 1) * 128]
                    rhs = wv[:, di * D:(di + 1) * D]
                    nc.tensor.matmul(ps[:, 0:512], lhsT, rhs[:, 0:512],
                                     start=(di == 0), stop=(di == ND - 1))
                    nc.tensor.matmul(ps[:, 512:1024], lhsT, rhs[:, 512:1024],
                                     start=(di == 0), stop=(di == ND - 1))
                vr = va_r[:, st]
                nc.vector.tensor_copy(
                    vr[:, :, 0:64], ps.rearrange("p (h c) -> p h c", c=64))

            # ---------------- qk projection + PoPE ----------------
            q2_all = pp.tile([128, H * S], bf, tag="q2", bufs=1)
            k2_all = pp.tile([128, H * S], bf, tag="k2", bufs=1)
            if attn_only:
                nc.vector.memset(q2_all[:, 0:S], 0.01)
                nc.vector.memset(k2_all[:, 0:S], 0.01)
                nc.vector.memset(vaug[:, 0:H * 65], 0.01)
            for is_k in ((False, True) if not attn_only else ()):
                wqk = load_w(D if is_k else 0)
                dest = k2_all if is_k else q2_all
                cosr = r_ck if is_k else r_cq
                sinr = r_sk if is_k else r_sq
                for j in range(8):
                    ps = psp.tile([128, S], f32, tag="ps_proj", bufs=2)
                    for di in range(ND):
                        lhsT = wqk[:, di * D + j * 128:di * D + (j + 1) * 128]
                        rhs = hsT[:, di * S:(di + 1) * S]
                        nc.tensor.matmul(ps[:, 0:512], lhsT, rhs[:, 0:512],
                                         start=(di == 0), stop=(di == ND - 1))
                        nc.tensor.matmul(ps[:, 512:1024], lhsT, rhs[:, 512:1024],
                                         start=(di == 0), stop=(di == ND - 1))
                    # softplus(x) = ln(exp(x) + 1) — same ACT table as Exp
                    pse = wp.tile([128, S], bf, tag="sb_exp", bufs=2)
                    nc.scalar.activation(pse[:], ps[:], AF.Exp)
                    mu = wp.tile([128, S], bf, tag="mu", bufs=2)
                    nc.scalar.activation(mu[:], pse[:], AF.Ln, bias=1.0)
                    for hh in range(2):
                        h = 2 * j + hh
                        lo, hi = hh * 64, hh * 64 + 64
                        dsl = dest[:, h * S:(h + 1) * S]
                        nc.vector.tensor_mul(dsl[0:64, :], mu[lo:hi, :], cosr[lo:hi, :])
                        nc.vector.tensor_mul(dsl[64:128, :], mu[lo:hi, :], sinr[lo:hi, :])

        # ---------------- attention per head ----------------
        attnT = pp.tile([128, 8 * S], bf, tag="attnT", bufs=1)
        if skip_attn:
            nc.vector.memset(attnT[:, 0:8 * S], 0.01)
        psa = tc.tile_pool(name="psum_attn", bufs=1, space="PSUM")
        psp = psa.__enter__()

        def emit_scores(q2, k2, kt):
            # scoresT[j, i] for one k-tile
            pss = psp.tile([128, S], f32, tag="ps_s", bufs=2)
            k2s = k2[:, kt * 128:(kt + 1) * 128]
            nc.tensor.matmul(pss[:, 0:512], k2s, q2[:, 0:512],
                             start=True, stop=True)
            nc.tensor.matmul(pss[:, 512:1024], k2s, q2[:, 512:1024],
                             start=True, stop=True)
            return pss

        def emit_norm(h, pv):
            # normalize head h: bcast 1/denom over partitions via K=1
            # f32r matmul, then scale attnT rows at PSUM evacuation
            rc = wp.tile([1, S], f32, tag="recip", bufs=2)
            nc.vector.reciprocal(rc[:], pv[64:65, :])
            rcr = wp.tile([1, S], dt.float32r, tag="recip_r", bufs=2)
            nc.vector.tensor_copy(rcr[:], rc[:])
            pbc = psp.tile([64, S], f32, tag="ps_bc", bufs=1)
            nc.tensor.matmul(pbc[:, 0:512], ones_r[:], rcr[:, 0:512],
                             start=True, stop=True)
            nc.tensor.matmul(pbc[:, 512:1024], ones_r[:], rcr[:, 512:1024],
                             start=True, stop=True)
            bc = wp.tile([64, S], f32, tag="bc_sb", bufs=2)
            nc.vector.tensor_copy(bc[:], pbc[:])
            dsl = attnT[(h % 2) * 64:(h % 2) * 64 + 64,
                        (h // 2) * S:(h // 2 + 1) * S]
            nc.vector.tensor_mul(dsl, pv[0:64, :], bc[:])

        # Software-pipelined emission: scores run one k-tile ahead of
        # exp+PV so the in-order PE never waits on ACT; the previous
        # head's normalization chain is deferred into the next head.
        prev = None  # (h, pv) awaiting normalization
        for h in range(H if not skip_attn else 0):
            hs_ = 0 if attn_only else h
            q2 = q2_all[:, hs_ * S:(hs_ + 1) * S]
            k2 = k2_all[:, hs_ * S:(hs_ + 1) * S]
            pv = psp.tile([65, S], f32, tag="ps_pv", bufs=1)
            pss = emit_scores(q2, k2, 0)
            for kt in range(NT):
                pss_next = emit_scores(q2, k2, kt + 1) if kt + 1 < NT else None
                pt = wp.tile([128, S], bf, tag="probsT", bufs=3)
                nc.scalar.activation(pt[:], pss[:], AF.Exp, scale=SCALE)
                kv = 0 if attn_only else kt
                hv = 0 if attn_only else h
                va = vaug[:, kv * H * 65 + hv * 65:kv * H * 65 + (hv + 1) * 65]
                nc.tensor.matmul(pv[:, 0:512], va, pt[:, 0:512],
                                 start=(kt == 0), stop=(kt == NT - 1))
                nc.tensor.matmul(pv[:, 512:1024], va, pt[:, 512:1024],
                                 start=(kt == 0), stop=(kt == NT - 1))
                pss = pss_next
                if kt == 0 and prev is not None:
                    emit_norm(*prev)
            prev = (h, pv)
        if prev is not None:
            emit_norm(*prev)

        psa.__exit__(None, None, None)

        # ---------------- output projection ----------------
        with (
            tc.tile_pool(name="tail", bufs=1) as tp,
            tc.tile_pool(name="psum_tail", bufs=1, space="PSUM") as psp,
        ):
            wo = tp.tile([128, ND * D], bf, tag="wout", bufs=1)
            for i in range(ND):
                nc.sync.dma_start(
                    wo[:, i * D:(i + 1) * D], t_woutT[i * 128:(i + 1) * 128, :])
            for st in range(NT):
                ps = psp.tile([128, D], f32, tag="ps_o", bufs=2)
                for et in range(ND):
                    lhsT = attnT[:, et * S + st * 128:et * S + (st + 1) * 128]
                    rhs = wo[:, et * D:(et + 1) * D]
                    nc.tensor.matmul(ps[:, 0:512], lhsT, rhs[:, 0:512],
                                     start=(et == 0), stop=(et == ND - 1))
                    nc.tensor.matmul(ps[:, 512:1024], lhsT, rhs[:, 512:1024],
                                     start=(et == 0), stop=(et == ND - 1))
                ot = tp.tile([128, D], f32, tag="out_sb", bufs=2)
                nc.vector.tensor_copy(ot[:], ps[:])
                nc.sync.dma_start(t_out[st * 128:(st + 1) * 128, :], ot[:])


def build_bass(reps=1, **emit_kw):
    import concourse.bass as bass  # noqa: F401
    import concourse.mybir as mybir
    import concourse.tile as tile
    from concourse import bacc

    dt = mybir.dt
    nc = bacc.Bacc("TRN2", target_bir_lowering=False, debug=False)
    t_hsT = nc.dram_tensor("hsT", [D, S], dt.bfloat16, kind="ExternalInput").ap()
    t_wqkvT = nc.dram_tensor("wqkvT", [D, 3 * D], dt.bfloat16, kind="ExternalInput").ap()
    t_woutT = nc.dram_tensor("woutT", [D, D], dt.bfloat16, kind="ExternalInput").ap()
    t_rot = nc.dram_tensor("rot", [128, 4 * S], dt.bfloat16, kind="ExternalInput").ap()
    t_out = nc.dram_tensor("out", [S, D], dt.float32, kind="ExternalOutput").ap()
    with tile.TileContext(nc) as tc:
        if reps == 1:
            _emit(tc, nc, t_hsT, t_wqkvT, t_woutT, t_rot, t_out, **emit_kw)
        else:
            with tc.For_i(0, reps, 1):
                _emit(tc, nc, t_hsT, t_wqkvT, t_woutT, t_rot, t_out, **emit_kw)
    nc.compile()
    return nc


def make_in_maps(hidden_states, pos_xyz, w_qkv, w_out, phase_bias):
    hsT, wqkvT, woutT, cq, sq, ck, sk = _host_prep(
        hidden_states, pos_xyz, w_qkv, w_out, phase_bias)
    rot = np.concatenate([cq, sq, ck, sk], axis=2)  # [B, 128, 4*S]
    return [
        {
            "hsT": np.ascontiguousarray(hsT[b]),
            "wqkvT": wqkvT,
            "woutT": woutT,
            "rot": np.ascontiguousarray(rot[b]),
        }
        for b in range(B)
    ]


def kernel(hidden_states, attention_mask, pos_xyz, w_qkv, w_out, phase_bias):
    from concourse.bass_utils import run_bass_kernel_spmd

    in_maps = make_in_maps(hidden_states, pos_xyz, w_qkv, w_out, phase_bias)
    nc = build_bass()
    res = run_bass_kernel_spmd(nc, in_maps, list(range(B)))
    out = np.stack([np.asarray(res.results[c]["out"]) for c in range(B)])
    return out.astype(np.float32)



# revision 8
# speedup vs baseline: 744.3404x; 744.3404x over previous
"""Trainium2 Bass kernel for 3D-PoPE multi-head self-attention.

Sharding: pure data-parallel over batch (B=8 -> 8 cores, one batch element
per core). Weights replicated. All rotary/cache gathers precomputed on host
(tiny tensors); all matmuls/softmax on device.

Per-core device computation (S=1024, D=1024, H=16, HD=64):
  phase 1: v projection -> vaug (ones-augmented, per-(kt,head) [128,65])
  phase 2: q/k projection feature-major; softplus via batched Exp then
           batched Ln (one ACT table load per function run, not per tile);
           q2 = [mu*cos_t; mu*sin_t], k2 = [mu*cos_k; mu*sin_k]
  phase 3: per head: scoresT = k2_tile.T @ q2 (2 tiles in flight),
           probsT = exp(scoresT * scale), PV with ones-row denominator;
           pv double-buffered so the norm chain (reciprocal_approx_fast +
           gpsimd partition_broadcast + DVE mul) stays off the PE path
  phase 4: out = attnT.T @ w_out.T
"""
import math

import numpy as np
import ml_dtypes

B, S, D, H = 8, 1024, 1024, 16
HD = D // H
DX = HD // 3
DY = HD // 3
DZ = HD - DX - DY
MX, MY, MZ = 32, 32, 8
BASE = 10000.0
TWO_PI = 2.0 * math.pi
BF16 = ml_dtypes.bfloat16

NT = S // 128          # 8 sequence tiles
ND = D // 128          # 8 contraction tiles
SCALE = 1.0 / math.sqrt(2.0 * HD)


def _host_prep(hidden_states, pos_xyz, w_qkv, w_out, phase_bias):
    """Host-side: transposes, dtype casts, rotary cache gather."""
    def cache(dim, maxp):
        inv = 1.0 / (BASE ** (np.arange(dim, dtype=np.float64) / dim))
        t = np.arange(maxp, dtype=np.float64)[:, None] * inv[None, :]
        return np.cos(t), np.sin(t)

    cx, sx = cache(DX, MX)
    cy, sy = cache(DY, MY)
    cz, sz = cache(DZ, MZ)
    pos = np.asarray(pos_xyz)
    px = np.clip(pos[..., 0], 0, MX - 1).astype(np.int64)
    py = np.clip(pos[..., 1], 0, MY - 1).astype(np.int64)
    pz = np.clip(pos[..., 2], 0, MZ - 1).astype(np.int64)
    cos_t = np.concatenate([cx[px], cy[py], cz[pz]], axis=-1)  # [B,S,HD] f64
    sin_t = np.concatenate([sx[px], sy[py], sz[pz]], axis=-1)
    bias = np.clip(np.asarray(phase_bias, np.float64), -TWO_PI, 0.0)
    cos_b = np.cos(bias)
    sin_b = np.sin(bias)
    cos_k = cos_t * cos_b - sin_t * sin_b
    sin_k = sin_t * cos_b + cos_t * sin_b

    def dup(x):
        # [B,S,HD] -> [B, 128, S] bf16 with rows 0:64 == rows 64:128
        xt = np.ascontiguousarray(x.transpose(0, 2, 1))  # [B, HD, S]
        return np.concatenate([xt, xt], axis=1).astype(BF16)

    hs = np.asarray(hidden_states, np.float32)
    hsT = np.ascontiguousarray(hs.transpose(0, 2, 1)).astype(BF16)  # [B, D, S]
    wqkvT = np.ascontiguousarray(np.asarray(w_qkv, np.float32).T).astype(BF16)
    woutT = np.ascontiguousarray(np.asarray(w_out, np.float32).T).astype(BF16)
    return hsT, wqkvT, woutT, dup(cos_t), dup(sin_t), dup(cos_k), dup(sin_k)


def _emit(tc, nc, t_hsT, t_wqkvT, t_woutT, t_rot, t_out):
    import concourse.mybir as mybir

    dt = mybir.dt
    AF = mybir.ActivationFunctionType
    f32 = dt.float32
    bf = dt.bfloat16

    with tc.tile_pool(name="persist", bufs=1) as pp:
        vaug = pp.tile([128, NT * H * 65], bf, tag="vaug", bufs=1)
        va_r = vaug.rearrange("p (k h c) -> p k h c", k=NT, h=H)
        nc.vector.memset(va_r[:, :, :, 64:65], 1.0)
        q2_all = pp.tile([128, H * S], bf, tag="q2", bufs=1)
        k2_all = pp.tile([128, H * S], bf, tag="k2", bufs=1)
        attnT = pp.tile([128, 8 * S], bf, tag="attnT", bufs=1)

        # ---------------- projections ----------------
        with (
            tc.tile_pool(name="proj", bufs=1) as jp,
            tc.tile_pool(name="psum_proj", bufs=1, space="PSUM") as psp,
        ):
            rot = jp.tile([128, 4 * S], bf, tag="rot", bufs=1)
            nc.sync.dma_start(rot[:], t_rot[:])
            r_cq = rot[:, 0:S]
            r_sq = rot[:, S:2 * S]
            r_ck = rot[:, 2 * S:3 * S]
            r_sk = rot[:, 3 * S:4 * S]

            hsT = jp.tile([128, ND * S], bf, tag="hsT", bufs=1)
            for i in range(ND):
                nc.sync.dma_start(
                    hsT[:, i * S:(i + 1) * S], t_hsT[i * 128:(i + 1) * 128, :])

            def load_w(col0):
                w = jp.tile([128, ND * D], bf, tag="wq", bufs=2)
                for i in range(ND):
                    nc.sync.dma_start(
                        w[:, i * D:(i + 1) * D],
                        t_wqkvT[i * 128:(i + 1) * 128, col0:col0 + D])
                return w

            # v projection: lhsT = hsT block, rhs = wv -> psum [s, e]
            wv = load_w(2 * D)
            for st in range(NT):
                ps = psp.tile([128, D], f32, tag="ps_proj", bufs=2)
                for di in range(ND):
                    lhsT = hsT[:, di * S + st * 128:di * S + (st + 1) * 128]
                    rhs = wv[:, di * D:(di + 1) * D]
                    nc.tensor.matmul(ps[:, 0:512], lhsT, rhs[:, 0:512],
                                     start=(di == 0), stop=(di == ND - 1))
                    nc.tensor.matmul(ps[:, 512:1024], lhsT, rhs[:, 512:1024],
                                     start=(di == 0), stop=(di == ND - 1))
                vr = va_r[:, st]
                nc.vector.tensor_copy(
                    vr[:, :, 0:64], ps.rearrange("p (h c) -> p h c", c=64))

            # q/k projection feature-major: lhsT = w block, rhs = hsT.
            # Softplus = Ln(Exp(x) + 1); batch all Exps of one direction,
            # then all Lns, so the ACT table set switches O(1) times.
            pse = jp.tile([128, 8 * S], bf, tag="pse", bufs=1)
            for is_k in (False, True):
                wqk = load_w(D if is_k else 0)
                dest = k2_all if is_k else q2_all
                cosr = r_ck if is_k else r_cq
                sinr = r_sk if is_k else r_sq
                for j in range(8):
                    ps = psp.tile([128, S], f32, tag="ps_proj", bufs=2)
                    for di in range(ND):
                        lhsT = wqk[:, di * D + j * 128:di * D + (j + 1) * 128]
                        rhs = hsT[:, di * S:(di + 1) * S]
                        nc.tensor.matmul(ps[:, 0:512], lhsT, rhs[:, 0:512],
                                         start=(di == 0), stop=(di == ND - 1))
                        nc.tensor.matmul(ps[:, 512:1024], lhsT,
                                         rhs[:, 512:1024],
                                         start=(di == 0), stop=(di == ND - 1))
                    nc.scalar.activation(pse[:, j * S:(j + 1) * S], ps[:],
                                         AF.Exp)
                for j in range(8):
                    mu = jp.tile([128, S], bf, tag="mu", bufs=2)
                    nc.scalar.activation(mu[:], pse[:, j * S:(j + 1) * S],
                                         AF.Ln, bias=1.0)
                    for hh in range(2):
                        h = 2 * j + hh
                        lo, hi = hh * 64, hh * 64 + 64
                        dsl = dest[:, h * S:(h + 1) * S]
                        nc.vector.tensor_mul(dsl[0:64, :], mu[lo:hi, :],
                                             cosr[lo:hi, :])
                        nc.vector.tensor_mul(dsl[64:128, :], mu[lo:hi, :],
                                             sinr[lo:hi, :])

        # ---------------- attention per head ----------------
        with (
            tc.tile_pool(name="attn", bufs=1) as wp,
            tc.tile_pool(name="psum_attn", bufs=1, space="PSUM") as psa,
        ):
            def emit_scores(h, kt):
                pss = psa.tile([128, S], f32, tag="ps_s", bufs=2)
                q2 = q2_all[:, h * S:(h + 1) * S]
                k2s = k2_all[:, h * S + kt * 128:h * S + (kt + 1) * 128]
                nc.tensor.matmul(pss[:, 0:512], k2s, q2[:, 0:512],
                                 start=True, stop=True)
                nc.tensor.matmul(pss[:, 512:1024], k2s, q2[:, 512:1024],
                                 start=True, stop=True)
                return pss

            def emit_norm(h, pv):
                # 1/denominator broadcast over 64 partitions without
                # touching the PE: fast DVE reciprocal, gpsimd broadcast,
                # then one DVE mul writing the normalized attnT slice.
                # denom sits on PSUM partition 64; custom-DVE recip needs
                # aligned partitions, so shift-copy it to partition 0 first
                dn = wp.tile([1, S], f32, tag="dn", bufs=2)
                nc.vector.tensor_copy(dn[:], pv[64:65, :])
                rc = wp.tile([1, S], f32, tag="rc", bufs=2)
                nc.vector.reciprocal_approx_fast(rc[:], dn[:])
                bc = wp.tile([64, S], f32, tag="bc", bufs=2)
                nc.gpsimd.partition_broadcast(bc[:], rc[:], channels=64)
                dsl = attnT[(h % 2) * 64:(h % 2) * 64 + 64,
                            (h // 2) * S:(h // 2 + 1) * S]
                nc.vector.tensor_mul(dsl, pv[0:64, :], bc[:])

            for h in range(H):
                pv = psa.tile([65, S], f32, tag="ps_pv", bufs=2)
                window = [emit_scores(h, 0), emit_scores(h, 1)]
                for kt in range(NT):
                    pss = window.pop(0)
                    pt = wp.tile([128, S], bf, tag="pt", bufs=4)
                    nc.scalar.activation(pt[:], pss[:], AF.Exp, scale=SCALE)
                    va = vaug[:, kt * H * 65 + h * 65:kt * H * 65 + (h + 1) * 65]
                    nc.tensor.matmul(pv[:, 0:512], va, pt[:, 0:512],
                                     start=(kt == 0), stop=(kt == NT - 1))
                    nc.tensor.matmul(pv[:, 512:1024], va, pt[:, 512:1024],
                                     start=(kt == 0), stop=(kt == NT - 1))
                    if kt + 2 < NT:
                        window.append(emit_scores(h, kt + 2))
                emit_norm(h, pv)

        # ---------------- output projection ----------------
        with (
            tc.tile_pool(name="tail", bufs=1) as tp,
            tc.tile_pool(name="psum_tail", bufs=1, space="PSUM") as psp,
        ):
            wo = tp.tile([128, ND * D], bf, tag="wout", bufs=1)
            for i in range(ND):
                nc.sync.dma_start(
                    wo[:, i * D:(i + 1) * D], t_woutT[i * 128:(i + 1) * 128, :])
            for st in range(NT):
                ps = psp.tile([128, D], f32, tag="ps_o", bufs=2)
                for et in range(ND):
                    lhsT = attnT[:, et * S + st * 128:et * S + (st + 1) * 128]
                    rhs = wo[:, et * D:(et + 1) * D]
                    nc.tensor.matmul(ps[:, 0:512], lhsT, rhs[:, 0:512],
                                     start=(et == 0), stop=(et == ND - 1))
                    nc.tensor.matmul(ps[:, 512:1024], lhsT, rhs[:, 512:1024],
                                     start=(et == 0), stop=(et == ND - 1))
                ot = tp.tile([128, D], f32, tag="out_sb", bufs=2)
                nc.vector.tensor_copy(ot[:], ps[:])
                nc.sync.dma_start(t_out[st * 128:(st + 1) * 128, :], ot[:])


def build_bass(reps=1, **emit_kw):
    import concourse.bass as bass  # noqa: F401
    import concourse.mybir as mybir
    import concourse.tile as tile
    from concourse import bacc

    dt = mybir.dt
    nc = bacc.Bacc("TRN2", target_bir_lowering=False, debug=False)
    t_hsT = nc.dram_tensor("hsT", [D, S], dt.bfloat16, kind="ExternalInput").ap()
    t_wqkvT = nc.dram_tensor("wqkvT", [D, 3 * D], dt.bfloat16, kind="ExternalInput").ap()
    t_woutT = nc.dram_tensor("woutT", [D, D], dt.bfloat16, kind="ExternalInput").ap()
    t_rot = nc.dram_tensor("rot", [128, 4 * S], dt.bfloat16, kind="ExternalInput").ap()
    t_out = nc.dram_tensor("out", [S, D], dt.float32, kind="ExternalOutput").ap()
    with tile.TileContext(nc) as tc:
        if reps == 1:
            _emit(tc, nc, t_hsT, t_wqkvT, t_woutT, t_rot, t_out, **emit_kw)
        else:
            with tc.For_i(0, reps, 1):
                _emit(tc, nc, t_hsT, t_wqkvT, t_woutT, t_rot, t_out, **emit_kw)
    nc.compile()
    return nc


def make_in_maps(hidden_states, pos_xyz, w_qkv, w_out, phase_bias):
    hsT, wqkvT, woutT, cq, sq, ck, sk = _host_prep(
        hidden_states, pos_xyz, w_qkv, w_out, phase_bias)
    rot = np.concatenate([cq, sq, ck, sk], axis=2)  # [B, 128, 4*S]
    return [
        {
            "hsT": np.ascontiguousarray(hsT[b]),
            "wqkvT": wqkvT,
            "woutT": woutT,
            "rot": np.ascontiguousarray(rot[b]),
        }
        for b in range(B)
    ]


def kernel(hidden_states, attention_mask, pos_xyz, w_qkv, w_out, phase_bias):
    from concourse.bass_utils import run_bass_kernel_spmd

    in_maps = make_in_maps(hidden_states, pos_xyz, w_qkv, w_out, phase_bias)
    nc = build_bass()
    res = run_bass_kernel_spmd(nc, in_maps, list(range(B)))
    out = np.stack([np.asarray(res.results[c]["out"]) for c in range(B)])
    return out.astype(np.float32)


# revision 10
# speedup vs baseline: 813.3336x; 1.0927x over previous
"""Trainium2 Bass kernel for 3D-PoPE multi-head self-attention.

Sharding: pure data-parallel over batch (B=8 -> 8 cores, one batch element
per core). Weights replicated. All rotary/cache gathers precomputed on host
(tiny tensors); all matmuls/softmax on device.

Per-core device computation (S=1024, D=1024, H=16, HD=64):
  phase 1: v projection -> vaug (ones-augmented, per-(kt,head) [128,65])
  phase 2: q/k projection feature-major; softplus via batched Exp then
           batched Ln (one ACT table load per function run, not per tile);
           q2 = [mu*cos_t; mu*sin_t], k2 = [mu*cos_k; mu*sin_k]
  phase 3: per head: scoresT = k2_tile.T @ q2 (2 tiles in flight),
           probsT = exp(scoresT * scale), PV with ones-row denominator;
           pv double-buffered so the norm chain (reciprocal_approx_fast +
           gpsimd partition_broadcast + DVE mul) stays off the PE path
  phase 4: out = attnT.T @ w_out.T
"""
import math

import numpy as np
import ml_dtypes

B, S, D, H = 8, 1024, 1024, 16
HD = D // H
DX = HD // 3
DY = HD // 3
DZ = HD - DX - DY
MX, MY, MZ = 32, 32, 8
BASE = 10000.0
TWO_PI = 2.0 * math.pi
BF16 = ml_dtypes.bfloat16

NT = S // 128          # 8 sequence tiles
ND = D // 128          # 8 contraction tiles
SCALE = 1.0 / math.sqrt(2.0 * HD)


def _host_prep(hidden_states, pos_xyz, w_qkv, w_out, phase_bias):
    """Host-side: transposes, dtype casts, rotary cache gather."""
    def cache(dim, maxp):
        inv = 1.0 / (BASE ** (np.arange(dim, dtype=np.float64) / dim))
        t = np.arange(maxp, dtype=np.float64)[:, None] * inv[None, :]
        return np.cos(t), np.sin(t)

    cx, sx = cache(DX, MX)
    cy, sy = cache(DY, MY)
    cz, sz = cache(DZ, MZ)
    pos = np.asarray(pos_xyz)
    px = np.clip(pos[..., 0], 0, MX - 1).astype(np.int64)
    py = np.clip(pos[..., 1], 0, MY - 1).astype(np.int64)
    pz = np.clip(pos[..., 2], 0, MZ - 1).astype(np.int64)
    cos_t = np.concatenate([cx[px], cy[py], cz[pz]], axis=-1)  # [B,S,HD] f64
    sin_t = np.concatenate([sx[px], sy[py], sz[pz]], axis=-1)
    bias = np.clip(np.asarray(phase_bias, np.float64), -TWO_PI, 0.0)
    cos_b = np.cos(bias)
    sin_b = np.sin(bias)
    cos_k = cos_t * cos_b - sin_t * sin_b
    sin_k = sin_t * cos_b + cos_t * sin_b

    def dup(x):
        # [B,S,HD] -> [B, 128, S] bf16 with rows 0:64 == rows 64:128
        xt = np.ascontiguousarray(x.transpose(0, 2, 1))  # [B, HD, S]
        return np.concatenate([xt, xt], axis=1).astype(BF16)

    hs = np.asarray(hidden_states, np.float32)
    hsT = np.ascontiguousarray(hs.transpose(0, 2, 1)).astype(BF16)  # [B, D, S]
    wqkvT = np.ascontiguousarray(np.asarray(w_qkv, np.float32).T).astype(BF16)
    woutT = np.ascontiguousarray(np.asarray(w_out, np.float32).T).astype(BF16)
    return hsT, wqkvT, woutT, dup(cos_t), dup(sin_t), dup(cos_k), dup(sin_k)


def _emit(tc, nc, t_hsT, t_wqkvT, t_woutT, t_rot, t_out):
    import concourse.mybir as mybir

    dt = mybir.dt
    AF = mybir.ActivationFunctionType
    f32 = dt.float32
    bf = dt.bfloat16

    # Preload the combined exp+ln ACT table set once: Exp and Ln both live
    # in natural_log_exp_and_others, so no per-activation table thrash.
    tables = None
    try:
        from concourse.hw_specs import get_activation_tables
        tables = list(get_activation_tables(nc.m.arch).keys())
    except Exception:
        pass
    if tables and "natural_log_exp_and_others" in tables:
        nc.scalar.add_instruction(mybir.InstLoadActFuncSet(
            act_func_set_id=tables.index("natural_log_exp_and_others"),
            name=nc.get_next_instruction_name(), ins=[], outs=[]))

    with tc.tile_pool(name="persist", bufs=1) as pp:
        vaug = pp.tile([128, NT * H * 65], bf, tag="vaug", bufs=1)
        va_r = vaug.rearrange("p (k h c) -> p k h c", k=NT, h=H)
        nc.vector.memset(va_r[:, :, :, 64:65], 1.0)
        q2_all = pp.tile([128, H * S], bf, tag="q2", bufs=1)
        k2_all = pp.tile([128, H * S], bf, tag="k2", bufs=1)
        attnT = pp.tile([128, 8 * S], bf, tag="attnT", bufs=1)

        # ---------------- projections ----------------
        with (
            tc.tile_pool(name="proj", bufs=1) as jp,
            tc.tile_pool(name="psum_proj", bufs=1, space="PSUM") as psp,
        ):
            def load_w(col0):
                w = jp.tile([128, ND * D], bf, tag="wq", bufs=2)
                for i in range(ND):
                    nc.sync.dma_start(
                        w[:, i * D:(i + 1) * D],
                        t_wqkvT[i * 128:(i + 1) * 128, col0:col0 + D])
                return w

            # Interleave hsT and wv transfers so the first v-proj matmul
            # (needs hsT block 0 + wv block 0) starts ASAP; rot is not
            # needed until softplus, so it loads last.
            hsT = jp.tile([128, ND * S], bf, tag="hsT", bufs=1)
            wv = jp.tile([128, ND * D], bf, tag="wq", bufs=2)
            for i in range(ND):
                nc.sync.dma_start(
                    hsT[:, i * S:(i + 1) * S], t_hsT[i * 128:(i + 1) * 128, :])
                nc.sync.dma_start(
                    wv[:, i * D:(i + 1) * D],
                    t_wqkvT[i * 128:(i + 1) * 128, 2 * D:3 * D])

            rot = jp.tile([128, 4 * S], bf, tag="rot", bufs=1)
            nc.sync.dma_start(rot[:], t_rot[:])
            r_cq = rot[:, 0:S]
            r_sq = rot[:, S:2 * S]
            r_ck = rot[:, 2 * S:3 * S]
            r_sk = rot[:, 3 * S:4 * S]
            for st in range(NT):
                ps = psp.tile([128, D], f32, tag="ps_proj", bufs=2)
                for di in range(ND):
                    lhsT = hsT[:, di * S + st * 128:di * S + (st + 1) * 128]
                    rhs = wv[:, di * D:(di + 1) * D]
                    nc.tensor.matmul(ps[:, 0:512], lhsT, rhs[:, 0:512],
                                     start=(di == 0), stop=(di == ND - 1))
                    nc.tensor.matmul(ps[:, 512:1024], lhsT, rhs[:, 512:1024],
                                     start=(di == 0), stop=(di == ND - 1))
                vr = va_r[:, st]
                nc.vector.tensor_copy(
                    vr[:, :, 0:64], ps.rearrange("p (h c) -> p h c", c=64))

            # q/k projection feature-major: lhsT = w block, rhs = hsT.
            # Softplus = Ln(Exp(x) + 1); batch all Exps of one direction,
            # then all Lns, so the ACT table set switches O(1) times.
            pse = jp.tile([128, 8 * S], bf, tag="pse", bufs=1)
            for is_k in (False, True):
                wqk = load_w(D if is_k else 0)
                dest = k2_all if is_k else q2_all
                cosr = r_ck if is_k else r_cq
                sinr = r_sk if is_k else r_sq
                for j in range(8):
                    ps = psp.tile([128, S], f32, tag="ps_proj", bufs=2)
                    for di in range(ND):
                        lhsT = wqk[:, di * D + j * 128:di * D + (j + 1) * 128]
                        rhs = hsT[:, di * S:(di + 1) * S]
                        nc.tensor.matmul(ps[:, 0:512], lhsT, rhs[:, 0:512],
                                         start=(di == 0), stop=(di == ND - 1))
                        nc.tensor.matmul(ps[:, 512:1024], lhsT,
                                         rhs[:, 512:1024],
                                         start=(di == 0), stop=(di == ND - 1))
                    nc.scalar.activation(pse[:, j * S:(j + 1) * S], ps[:],
                                         AF.Exp)
                for j in range(8):
                    mu = jp.tile([128, S], bf, tag="mu", bufs=2)
                    nc.scalar.activation(mu[:], pse[:, j * S:(j + 1) * S],
                                         AF.Ln, bias=1.0)
                    for hh in range(2):
                        h = 2 * j + hh
                        lo, hi = hh * 64, hh * 64 + 64
                        dsl = dest[:, h * S:(h + 1) * S]
                        nc.vector.tensor_mul(dsl[0:64, :], mu[lo:hi, :],
                                             cosr[lo:hi, :])
                        nc.vector.tensor_mul(dsl[64:128, :], mu[lo:hi, :],
                                             sinr[lo:hi, :])

        # ---------------- attention per head ----------------
        with (
            tc.tile_pool(name="attn", bufs=1) as wp,
            tc.tile_pool(name="psum_attn", bufs=1, space="PSUM") as psa,
        ):
            def emit_scores(h, kt):
                pss = psa.tile([128, S], f32, tag="ps_s", bufs=2)
                q2 = q2_all[:, h * S:(h + 1) * S]
                k2s = k2_all[:, h * S + kt * 128:h * S + (kt + 1) * 128]
                nc.tensor.matmul(pss[:, 0:512], k2s, q2[:, 0:512],
                                 start=True, stop=True)
                nc.tensor.matmul(pss[:, 512:1024], k2s, q2[:, 512:1024],
                                 start=True, stop=True)
                return pss

            def emit_norm(h, pv):
                # 1/denominator broadcast over 64 partitions without
                # touching the PE: fast DVE reciprocal, gpsimd broadcast,
                # then one DVE mul writing the normalized attnT slice.
                # denom sits on PSUM partition 64; custom-DVE recip needs
                # aligned partitions, so shift-copy it to partition 0 first
                dn = wp.tile([1, S], f32, tag="dn", bufs=2)
                nc.vector.tensor_copy(dn[:], pv[64:65, :])
                rc = wp.tile([1, S], f32, tag="rc", bufs=2)
                nc.vector.reciprocal_approx_fast(rc[:], dn[:])
                bc = wp.tile([64, S], f32, tag="bc", bufs=2)
                nc.gpsimd.partition_broadcast(bc[:], rc[:], channels=64)
                dsl = attnT[(h % 2) * 64:(h % 2) * 64 + 64,
                            (h // 2) * S:(h // 2 + 1) * S]
                nc.vector.tensor_mul(dsl, pv[0:64, :], bc[:])

            for h in range(H):
                pv = psa.tile([65, S], f32, tag="ps_pv", bufs=2)
                window = [emit_scores(h, 0), emit_scores(h, 1)]
                for kt in range(NT):
                    pss = window.pop(0)
                    pt = wp.tile([128, S], bf, tag="pt", bufs=4)
                    nc.scalar.activation(pt[:], pss[:], AF.Exp, scale=SCALE)
                    va = vaug[:, kt * H * 65 + h * 65:kt * H * 65 + (h + 1) * 65]
                    nc.tensor.matmul(pv[:, 0:512], va, pt[:, 0:512],
                                     start=(kt == 0), stop=(kt == NT - 1))
                    nc.tensor.matmul(pv[:, 512:1024], va, pt[:, 512:1024],
                                     start=(kt == 0), stop=(kt == NT - 1))
                    if kt + 2 < NT:
                        window.append(emit_scores(h, kt + 2))
                emit_norm(h, pv)

        # ---------------- output projection ----------------
        with (
            tc.tile_pool(name="tail", bufs=1) as tp,
            tc.tile_pool(name="psum_tail", bufs=1, space="PSUM") as psp,
        ):
            wo = tp.tile([128, ND * D], bf, tag="wout", bufs=1)
            for i in range(ND):
                nc.sync.dma_start(
                    wo[:, i * D:(i + 1) * D], t_woutT[i * 128:(i + 1) * 128, :])
            for st in range(NT):
                ps = psp.tile([128, D], f32, tag="ps_o", bufs=2)
                for et in range(ND):
                    lhsT = attnT[:, et * S + st * 128:et * S + (st + 1) * 128]
                    rhs = wo[:, et * D:(et + 1) * D]
                    nc.tensor.matmul(ps[:, 0:512], lhsT, rhs[:, 0:512],
                                     start=(et == 0), stop=(et == ND - 1))
                    nc.tensor.matmul(ps[:, 512:1024], lhsT, rhs[:, 512:1024],
                                     start=(et == 0), stop=(et == ND - 1))
                ot = tp.tile([128, D], f32, tag="out_sb", bufs=2)
                nc.vector.tensor_copy(ot[:], ps[:])
                nc.sync.dma_start(t_out[st * 128:(st + 1) * 128, :], ot[:])


def build_bass(reps=1, **emit_kw):
    import concourse.bass as bass  # noqa: F401
    import concourse.mybir as mybir
    import concourse.tile as tile
    from concourse import bacc

    dt = mybir.dt
    nc = bacc.Bacc("TRN2", target_bir_lowering=False, debug=False)
    t_hsT = nc.dram_tensor("hsT", [D, S], dt.bfloat16, kind="ExternalInput").ap()
    t_wqkvT = nc.dram_tensor("wqkvT", [D, 3 * D], dt.bfloat16, kind="ExternalInput").ap()
    t_woutT = nc.dram_tensor("woutT", [D, D], dt.bfloat16, kind="ExternalInput").ap()
    t_rot = nc.dram_tensor("rot", [128, 4 * S], dt.bfloat16, kind="ExternalInput").ap()
    t_out = nc.dram_tensor("out", [S, D], dt.float32, kind="ExternalOutput").ap()
    with tile.TileContext(nc) as tc:
        if reps == 1:
            _emit(tc, nc, t_hsT, t_wqkvT, t_woutT, t_rot, t_out, **emit_kw)
        else:
            with tc.For_i(0, reps, 1):
                _emit(tc, nc, t_hsT, t_wqkvT, t_woutT, t_rot, t_out, **emit_kw)
    nc.compile()
    return nc


def make_in_maps(hidden_states, pos_xyz, w_qkv, w_out, phase_bias):
    hsT, wqkvT, woutT, cq, sq, ck, sk = _host_prep(
        hidden_states, pos_xyz, w_qkv, w_out, phase_bias)
    rot = np.concatenate([cq, sq, ck, sk], axis=2)  # [B, 128, 4*S]
    return [
        {
            "hsT": np.ascontiguousarray(hsT[b]),
            "wqkvT": wqkvT,
            "woutT": woutT,
            "rot": np.ascontiguousarray(rot[b]),
        }
        for b in range(B)
    ]


def kernel(hidden_states, attention_mask, pos_xyz, w_qkv, w_out, phase_bias):
    from concourse.bass_utils import run_bass_kernel_spmd

    in_maps = make_in_maps(hidden_states, pos_xyz, w_qkv, w_out, phase_bias)
    nc = build_bass()
    res = run_bass_kernel_spmd(nc, in_maps, list(range(B)))
    out = np.stack([np.asarray(res.results[c]["out"]) for c in range(B)])
    return out.astype(np.float32)
